# revision 1
# baseline (speedup 1.0000x reference)
"""Trainium2 Bass kernel for relative-position attention (nn_AttentionMechanism).

Math (per batch b):
  q,k,v = h@Wq, h@Wk, h@Wv  (biases are zero in this problem)
  scores[l,r] = (q[l].k[r] + q[l].E[l-r+1023] + k[r].E[l-r+1023]) / sqrt(64)
  out = softmax(scores) @ v @ Wd

Sharding: 8 cores = (batch b in 0..3) x (query half lh in 0..1).
Each core computes out rows [lh*512, lh*512+512) for batch b.

Per-core algorithm (T orientation: score tiles are [r partitions, l free]):
  - xT via PE transposes; qT/kT = W^T @ xT matmuls (scaled by 8^-1/4... see SCALE);
    v natural with a 64-wide ones block appended (gives softmax denominators for
    free as extra rows of the PV matmul output).
  - Relative-position terms need a diagonal "shear" gather E[l-r+1023], which no
    TRN2 engine can do on-chip (all gathers share indices per 16-partition group).
    Mechanism: music-transformer stride trick through DRAM:
      kd[r,j] = k[r].E_win[j] (fp16) written with row stride 640, read back
        with row stride 639 -> the read IS rel_k^T (plain HWDGE DMA).
      qd[l,u] = q[l].E_win_rev[u] (fp16) written with row stride 1536, read back
        with row stride 1535 through the HWDGE xbar transpose-DMA -> rel_q^T.
      rel_q+rel_k summed on GPSIMD (idle engine), then one DVE add from the
        content-score PSUM, exp on ScalarE.
  - exp on ScalarE (no max subtraction needed: |scores| <~ 1.5 by construction),
    PV + denominators on PE, per-head normalize, then out-projection.
"""

import sys

sys.path.insert(0, "/opt/trn_rl_repo")

import numpy as np

import concourse.bass as bass
import concourse.mybir as mybir
import concourse.tile as tile
from concourse import bacc
from concourse.bass_utils import run_bass_kernel_spmd

FP32 = mybir.dt.float32
FP16 = mybir.dt.float16
ADD = mybir.AluOpType.add
MULT = mybir.AluOpType.mult
EXP = mybir.ActivationFunctionType.Exp

N_CORES = 8
D, H, HD = 768, 12, 64
LQ, LK = 512, 1024
EW = 1536          # E window rows per core (= LQ + LK + pad)
KD_W = 640         # kd chunk width (639 used + 1 pad col)
QD_W = 1536        # qd row stride
SCALE = 0.35355339059327373  # 8**-0.5 applied to q,k AND E => all terms get /8


def _strided_view(ap, dims, extra_offset):
    """Return a copy of `ap` with its [step,count] pairs and offset replaced."""
    v = ap.copy()
    a = v.ap
    assert len(a) == len(dims), (a, dims)
    for i, d in enumerate(dims):
        a[i] = d
    v.ap = a
    v.offset = v.offset + extra_offset
    return v


def build_nc(repeats=1):
    nc = bacc.Bacc("TRN2", target_bir_lowering=False, debug=False,
                   num_devices=N_CORES)

    hq = nc.dram_tensor("hidden_q_T", [D, LQ], FP32, kind="ExternalInput").ap()
    hkv = nc.dram_tensor("hidden_kv_T", [D, LK], FP32, kind="ExternalInput").ap()
    wq = nc.dram_tensor("Wq", [D, D], FP32, kind="ExternalInput").ap()
    wk = nc.dram_tensor("Wk", [D, D], FP32, kind="ExternalInput").ap()
    wv = nc.dram_tensor("Wv", [D, D], FP32, kind="ExternalInput").ap()
    wd = nc.dram_tensor("Wd", [D, D], FP32, kind="ExternalInput").ap()
    demb = nc.dram_tensor("demb_win_T", [HD, EW], FP32, kind="ExternalInput").ap()
    dembr = nc.dram_tensor("demb_win_rev_T", [HD, EW], FP32, kind="ExternalInput").ap()
    out = nc.dram_tensor("out", [LQ, D], FP32, kind="ExternalOutput").ap()

    with tile.TileContext(nc) as tc:
        for r in range(repeats):
            qd_dram = nc.dram_tensor(f"qd_scratch{r}", [H, LQ, QD_W], FP16).ap()
            kd_dram = nc.dram_tensor(f"kd_scratch{r}", [H, 8, 128, KD_W], FP16).ap()
            _body(nc, tc, hq, hkv, wq, wk, wv, wd, demb, dembr, out,
                  qd_dram, kd_dram)
    nc.compile()
    return nc


def _body(nc, tc, hq, hkv, wq, wk, wv, wd, demb, dembr, out, qd_dram, kd_dram):
    with tc.tile_pool(name="const", bufs=1) as cp:
        ones_row = cp.tile([1, 64], FP32, tag="ones_row")
        nc.gpsimd.memset(ones_row[:, :], 1.0)

        eT = cp.tile([128, EW], FP32, tag="eT")    # rows 0:64 == 64:128 (replicated)
        erT = cp.tile([128, EW], FP32, tag="erT")
        kT = [cp.tile([128, LK], FP32, tag=f"kT{i}", name=f"kT{i}") for i in range(6)]
        qT = [cp.tile([128, LQ], FP32, tag=f"qT{i}", name=f"qT{i}") for i in range(6)]
        vv = [cp.tile([128, 780], FP32, tag=f"v{i}", name=f"v{i}") for i in range(8)]
        ctxT = [cp.tile([128, LQ], FP32, tag=f"ctxT{i}", name=f"ctxT{i}") for i in range(6)]

        # ---------------- Phase A+B: loads (host pre-transposed) + projections
        with tc.tile_pool(name="xt", bufs=1) as xp:
            xT = [xp.tile([128, LK], FP32, tag=f"xT{i}", name=f"xT{i}") for i in range(6)]
            xqT = [xp.tile([128, LQ], FP32, tag=f"xqT{i}", name=f"xqT{i}") for i in range(6)]

            for half in range(2):
                nc.sync.dma_start(out=eT[64 * half:64 * (half + 1), :], in_=demb[:, :])
                nc.sync.dma_start(out=erT[64 * half:64 * (half + 1), :], in_=dembr[:, :])
            for i in range(6):
                nc.sync.dma_start(out=xT[i][:, :], in_=hkv[128 * i:128 * (i + 1), :])
                nc.sync.dma_start(out=xqT[i][:, :], in_=hq[128 * i:128 * (i + 1), :])

            # projections
            with tc.tile_pool(name="wld", bufs=1) as wp, \
                 tc.tile_pool(name="psB", bufs=2, space="PSUM") as pb:
                for widx, (wdram, dst, rhs_tiles, n_tok) in enumerate((
                        (wk, kT, xT, LK), (wq, qT, xqT, LQ), (wv, None, xT, LK))):
                    wtiles = []
                    for kk in range(6):
                        wt = wp.tile([128, D], FP32, tag=f"w{kk}")
                        nc.sync.dma_start(out=wt[:, :], in_=wdram[128 * kk:128 * (kk + 1), :])
                        wtiles.append(wt)
                    if dst is not None:  # q/k: out is [D, n_tok] transposed
                        for m in range(6):
                            ps = pb.tile([128, LK], FP32, tag="projp")
                            for kk in range(6):
                                for nh in range(n_tok // 512):
                                    nc.tensor.matmul(
                                        ps[:, 512 * nh:512 * (nh + 1)],
                                        wtiles[kk][:, 128 * m:128 * (m + 1)],
                                        rhs_tiles[kk][:, 512 * nh:512 * (nh + 1)],
                                        start=(kk == 0), stop=(kk == 5))
                            nc.scalar.mul(dst[m][:, 0:n_tok], ps[:, 0:n_tok], SCALE)
                    else:  # v: natural [tok, D]
                        for r in range(8):
                            ps = pb.tile([128, D], FP32, tag="projp")
                            for kk in range(6):
                                for o, w in ((0, 512), (512, 256)):
                                    nc.tensor.matmul(
                                        ps[:, o:o + w],
                                        xT[kk][:, 128 * r:128 * (r + 1)],
                                        wtiles[kk][:, o:o + w],
                                        start=(kk == 0), stop=(kk == 5))
                            nc.gpsimd.memset(vv[r][:, :], 1.0)
                            vdst = vv[r][:, 0:D].rearrange("p (h e) -> p h e", e=64)
                            vdst = _strided_view(vdst, [vdst.ap[0], (65, 12), (1, 64)], 0)
                            nc.scalar.copy(vdst, ps[:, 0:D].rearrange(
                                "p (h e) -> p h e", e=64))

        # ---------------- Phase C: per-head attention ----------------
        with tc.tile_pool(name="psC", bufs=2, space="PSUM") as pc, \
             tc.tile_pool(name="psCS", bufs=3, space="PSUM") as pcs, \
             tc.tile_pool(name="psCTX", bufs=1, space="PSUM") as pctx, \
             tc.tile_pool(name="wkC", bufs=3) as wc, \
             tc.tile_pool(name="wkC3", bufs=8) as wc3:
            def _emit_writes(h):
                hc, hp = h // 2, h % 2
                hr = slice(64 * hp, 64 * (hp + 1))
                # kd chunks -> DRAM
                for Jp in range(4):
                    kd_sb = wc.tile([128, 2 * KD_W], FP16, tag="kd_sb")
                    for half in range(2):
                        J = 2 * Jp + half
                        w0 = 896 - 128 * J
                        kdp = pc.tile([128, KD_W], FP32, tag="kdqd")
                        lhsT = kT[hc][hr, 128 * J:128 * (J + 1)]
                        nc.tensor.matmul(kdp[:, 0:512], lhsT, eT[hr, w0:w0 + 512],
                                         start=True, stop=True)
                        nc.tensor.matmul(kdp[:, 512:KD_W], lhsT,
                                         eT[hr, w0 + 512:w0 + KD_W],
                                         start=True, stop=True)
                        nc.scalar.copy(kd_sb[:, KD_W * half:KD_W * (half + 1)],
                                       kdp[:, 0:KD_W])
                    kdw = _strided_view(kd_dram[h, 2 * Jp].unsqueeze(1),
                                        [(KD_W, 128), (128 * KD_W, 2), (1, KD_W)], 0)
                    nc.sync.dma_start(out=kdw, in_=kd_sb[:, :].rearrange(
                        "p (two w) -> p two w", two=2))

                # qd chunks -> DRAM (fp16, reversed window)
                for Ip in range(2):
                    qd_sb = wc.tile([128, 2, 1152], FP16, tag="qd_sb")
                    for half in range(2):
                        I = 2 * Ip + half
                        c0 = 384 - 128 * I
                        lhsT = qT[hc][hr, 128 * I:128 * (I + 1)]
                        qdpA = pc.tile([128, KD_W], FP32, tag="kdqd")
                        for o, w in ((0, 512), (512, 128)):
                            nc.tensor.matmul(qdpA[:, o:o + w], lhsT,
                                             erT[hr, c0 + o:c0 + o + w],
                                             start=True, stop=True)
                        nc.vector.tensor_copy(qd_sb[:, half, 0:KD_W], qdpA[:, :])
                        qdpB = pc.tile([128, 512], FP32, tag="kdqd")
                        nc.tensor.matmul(qdpB[:, :], lhsT,
                                         erT[hr, c0 + KD_W:c0 + KD_W + 512],
                                         start=True, stop=True)
                        nc.vector.tensor_copy(qd_sb[:, half, KD_W:1152], qdpB[:, :])
                    # rows of the I-pair: row step 1536, I-step = 128*1536 - 128
                    c0p = 384 - 256 * Ip
                    qdw = _strided_view(
                        qd_dram[h, 256 * Ip:256 * Ip + 128, c0p:c0p + 1152]
                        .unsqueeze(1),
                        [(QD_W, 128), (128 * QD_W - 128, 2), (1, 1152)], 0)
                    nc.sync.dma_start(out=qdw, in_=qd_sb[:, :, :].rearrange(
                        "p a b -> p (a b)").rearrange("p (a b) -> p a b", a=2))


            def _emit_scores(h):
                hc, hp = h // 2, h % 2
                hr = slice(64 * hp, 64 * (hp + 1))
                # scores + PV
                ctxp = pctx.tile([65, LQ], FP32, tag="ctxp")
                for J in range(8):
                    # early independent reads: rel_q (xbar transpose) + rel_k
                    rq_sb = wc3.tile([128, LQ], FP16, tag="rq_sb")
                    qdv = _strided_view(qd_dram[h], [(QD_W - 1, LQ), (1, 128)],
                                        512 + 128 * J)
                    nc.scalar.dma_start(out=rq_sb[:, :], in_=qdv, transpose=True)
                    if J % 2 == 0:
                        rk2_sb = wc3.tile([128, 2, LQ], FP16, tag="rk2_sb")
                        kdv = _strided_view(
                            kd_dram[h, J].unsqueeze(1),
                            [(KD_W - 1, 128), (128 * KD_W, 2), (1, LQ)], 127)
                        nc.sync.dma_start(out=rk2_sb[:, :, :], in_=kdv)
                    rk_sb = rk2_sb[:, J % 2, :]
                    rel_sb = wc3.tile([128, LQ], FP16, tag="rel_sb")
                    nc.gpsimd.tensor_tensor(rel_sb[:, :], rq_sb[:, :], rk_sb, ADD)
                    csp = pcs.tile([128, LQ], FP32, tag="csp")
                    nc.tensor.matmul(csp[:, :], kT[hc][hr, 128 * J:128 * (J + 1)],
                                     qT[hc][hr, :], start=True, stop=True)
                    s_sb = wc3.tile([128, LQ], FP32, tag="s_sb")
                    nc.vector.tensor_tensor(s_sb[:, :], csp[:, :], rel_sb[:, :], ADD)
                    p_sb = wc3.tile([128, LQ], FP32, tag="p_sb")
                    nc.scalar.activation(p_sb[:, :], s_sb[:, :], EXP)
                    # PV (rows 0:64) + denominators (rows 64:128) as two
                    # col-tiled concurrent matmuls sharing the rhs stream
                    nc.tensor.matmul(ctxp[:, :], vv[J][:, 65 * h:65 * h + 65],
                                     p_sb[:, :], start=(J == 0), stop=(J == 7))

                # normalize: ctxT_h = ctx' * (1/denom) broadcast over partitions
                recip = wc.tile([1, LQ], FP32, tag="recip")
                nc.vector.reciprocal(recip[:, :], ctxp[64:65, :])
                bcp = pcs.tile([64, LQ], FP32, tag="csp")
                nc.tensor.matmul(bcp[:, :], ones_row[:, :], recip[:, :],
                                 start=True, stop=True)
                bc_sb = wc.tile([64, LQ], FP32, tag="bc_sb")
                nc.scalar.copy(bc_sb[:, :], bcp[:, :])
                nc.vector.tensor_tensor(ctxT[hc][hr, :], ctxp[0:64, :],
                                        bc_sb[:, :], MULT)


            for h in range(H + 1):
                if h < H:
                    _emit_writes(h)
                if h >= 1:
                    _emit_scores(h - 1)

        # ---------------- Phase D: output projection ----------------
        with tc.tile_pool(name="wdld", bufs=1) as dp, \
             tc.tile_pool(name="psD", bufs=2, space="PSUM") as pd, \
             tc.tile_pool(name="oD", bufs=2) as od:
            wdt = []
            for kk in range(6):
                wt = dp.tile([128, D], FP32, tag=f"wd{kk}")
                nc.sync.dma_start(out=wt[:, :], in_=wd[128 * kk:128 * (kk + 1), :])
                wdt.append(wt)
            for lc in range(4):
                ps = pd.tile([128, D], FP32, tag="outp")
                for kk in range(6):
                    for o, w in ((0, 512), (512, 256)):
                        nc.tensor.matmul(ps[:, o:o + w],
                                         ctxT[kk][:, 128 * lc:128 * (lc + 1)],
                                         wdt[kk][:, o:o + w],
                                         start=(kk == 0), stop=(kk == 5))
                o_sb = od.tile([128, D], FP32, tag="o_sb")
                nc.scalar.copy(o_sb[:, :], ps[:, :])
                nc.sync.dma_start(out=out[128 * lc:128 * (lc + 1), :], in_=o_sb[:, :])


_NC_CACHE = None


def _get_nc():
    global _NC_CACHE
    if _NC_CACHE is None:
        _NC_CACHE = build_nc()
    return _NC_CACHE


def make_in_maps(hidden_states, Wq, Wk, Wv, Wd, dist_emb):
    E = np.ascontiguousarray(np.asarray(dist_emb, np.float32))
    in_maps = []
    for core in range(N_CORES):
        b, lh = core // 2, core % 2
        l0 = LQ * lh
        win = np.zeros((EW, HD), np.float32)
        n = min(EW, E.shape[0] - l0)
        win[:n] = E[l0:l0 + n]
        wins = win * np.float32(SCALE)
        in_maps.append({
            "hidden_q_T": np.ascontiguousarray(hidden_states[b, l0:l0 + LQ].T),
            "hidden_kv_T": np.ascontiguousarray(hidden_states[b].T),
            "Wq": np.ascontiguousarray(Wq), "Wk": np.ascontiguousarray(Wk),
            "Wv": np.ascontiguousarray(Wv), "Wd": np.ascontiguousarray(Wd),
            "demb_win_T": np.ascontiguousarray(wins.T),
            "demb_win_rev_T": np.ascontiguousarray(wins[::-1].T),
        })
    return in_maps


def run(inputs, trace=False):
    """Returns (full_output [4,1024,768], BassKernelResults)."""
    nc = _get_nc()
    in_maps = make_in_maps(inputs["hidden_states"], inputs["Wq"], inputs["Wk"],
                           inputs["Wv"], inputs["Wd"], inputs["dist_emb"])
    res = run_bass_kernel_spmd(nc, in_maps, list(range(N_CORES)), trace=trace)
    full = np.zeros((4, LK, D), np.float32)
    for core in range(N_CORES):
        b, lh = core // 2, core % 2
        full[b, LQ * lh:LQ * (lh + 1)] = res.results[core]["out"]
    return full, res


def kernel(**inputs):
    full, _ = run(inputs, trace=False)
    return full


if __name__ == "__main__":
    # quick self-build check
    nc = build_nc()
    print("built ok")

